# revision 22
# baseline (speedup 1.0000x reference)
"""Trainium2 8-core kernel for nn_ACCSLP_59485297050024.

The reference is a multiplicative-update NMF-style solver on N=4096 nodes with
rank R=128 and N_ITERS=2, returning a scalar objective O.

Because U, H, W, V are initialized to all-ones (per the problem's input spec),
every multiplicative update keeps each factor CONSTANT along the rank axis, so
the whole computation collapses exactly to rank-1 vector recurrences:

    u1 = (rowsum(S) + b*rowsum(Z)) * 2/(3R)
    h1 = (S + a*X)^T (1/e1) / R,  e1 = u1 + a       v1 = Z^T (1/u1) / R
    w1 = X (1/h1) / R,   u2 = (S + b*Z)(1/d1) / R,  d1 = h1 + b*v1
    h2 = (S + a*X)^T (1/e2) / R,  e2 = u2 + a*w1    v2 = Z^T (1/u2) / R
    w2 = X (1/h2) / R
    O  = R[Su2 Sh2 + a Sw2 Sh2 + b Su2 Sv2]
         - (sum(S) + a sum(X) + b sum(Z)) log R
         - <log u2, rsS + b rsZ> - a <log w2, rsX>
         - <log h2, csS + a csX> - b <log v2, csZ>

(verified exact vs the reference, rel err ~2e-16 in float64).

Device strategy (8 NeuronCores): row-shard S/X/Z (512 rows/core) and keep BOTH
the row-major shard and its transpose resident in SBUF (bf16, 2x96KB/partition).
All contractions are TensorE matmuls with tiny stationary vectors:
  - "B" passes (contract over rows) use the row-major tiles; partial results
    are AllReduce-summed across cores (2 AllReduces total).
  - "A" passes (contract over cols) use the transposed tiles; results stay
    core-local (each core owns its 512 rows of u/w).
The final scalar assembly (logs + dot products on 4096-vectors) runs on host.
"""

import numpy as np
import ml_dtypes

N = 4096
R = 128
ALPHA = 0.5
BETA = 0.5
N_CORES = 8
RPC = N // N_CORES          # rows per core = 512
RG = RPC // 128             # row groups per core = 4
NC_CH = N // 128            # 128-column chunks = 32
NJ8 = N // 512              # 512-column chunks = 8

_CACHED = {}


def _build():
    import concourse.mybir as mybir
    import concourse.tile as tile
    from concourse import bacc
    from concourse.masks import make_identity

    bf16 = mybir.dt.bfloat16
    f32 = mybir.dt.float32

    nc = bacc.Bacc("TRN2", target_bir_lowering=False, debug=False,
                   num_devices=N_CORES, dynamic_dma_scratch_size=8192)

    # per-core external I/O
    sr_e = nc.declare_dram_parameter("sr", [128, RG, N], bf16, isOutput=False)
    xr_e = nc.declare_dram_parameter("xr", [128, RG, N], bf16, isOutput=False)
    zr_e = nc.declare_dram_parameter("zr", [128, RG, N], bf16, isOutput=False)
    sc_e = nc.declare_dram_parameter("sc", [128, NC_CH, RPC], bf16, isOutput=False)
    xc_e = nc.declare_dram_parameter("xc", [128, NC_CH, RPC], bf16, isOutput=False)
    zc_e = nc.declare_dram_parameter("zc", [128, NC_CH, RPC], bf16, isOutput=False)
    out_u1 = nc.declare_dram_parameter("u1", [1, RPC], f32, isOutput=True)
    out_u2 = nc.declare_dram_parameter("u2", [1, RPC], f32, isOutput=True)
    out_w2 = nc.declare_dram_parameter("w2", [1, RPC], f32, isOutput=True)
    out_rsx = nc.declare_dram_parameter("rsx", [1, RPC], f32, isOutput=True)
    out_h2 = nc.declare_dram_parameter("h2", [NC_CH, 128], f32, isOutput=True)
    out_v2 = nc.declare_dram_parameter("v2", [NC_CH, 128], f32, isOutput=True)
    out_cssx = nc.declare_dram_parameter("cssx", [NC_CH, 128], f32, isOutput=True)
    out_csz = nc.declare_dram_parameter("csz", [NC_CH, 128], f32, isOutput=True)

    ar1_out = nc.dram_tensor("ar1_out", [4, NC_CH, 128], f32, addr_space="Shared")
    ar2_out = nc.dram_tensor("ar2_out", [2, NC_CH, 128], f32, addr_space="Shared")
    groups = [list(range(N_CORES))]

    with tile.TileContext(nc) as tc:
        with (
            tc.tile_pool(name="big", bufs=1) as big,
            tc.tile_pool(name="small", bufs=1) as small,
            tc.tile_pool(name="stg", bufs=2) as stg,
            tc.tile_pool(name="pp", bufs=1, space="PSUM") as pp,
            tc.tile_pool(name="pstream", bufs=2, space="PSUM") as pstream,
            tc.tile_pool(name="ptrans", bufs=1, space="PSUM") as ptrans,
            tc.tile_pool(name="pwarm", bufs=1, space="PSUM") as pwarm,
            tc.tile_pool(name="dram", bufs=1, space="DRAM") as dram,
        ):
            # ---------- cross-core barrier first ----------
            # dummy AllReduce: absorbs per-execution collective warmup + core
            # start skew while the DMA loads run. Input is never written
            # (garbage) and output never read -- must not wait on anything.
            sync_in = nc.dram_tensor("sync_in", [1, 128], f32)
            sync_out = nc.dram_tensor("sync_out", [1, 128], f32, addr_space="Shared")
            with tc.high_priority():
                nc.gpsimd.collective_compute(
                    "AllReduce", mybir.AluOpType.add, replica_groups=groups,
                    ins=[sync_in[:].opt()], outs=[sync_out[:].opt()])

            # ---------- resident loads (pieces, ordered for pipelining) ----------
            QC = 8   # col-tile pieces per matrix (4 chunks each)
            QR = 8   # row-tile pieces per matrix (512 cols each = one B chunk)
            tCs4 = [big.tile([128, 4, RPC], bf16, name=f"tC_s{q}", tag=f"tC_s{q}") for q in range(QC)]
            tCz4 = [big.tile([128, 4, RPC], bf16, name=f"tC_z{q}", tag=f"tC_z{q}") for q in range(QC)]
            tCx4 = [big.tile([128, 4, RPC], bf16, name=f"tC_x{q}", tag=f"tC_x{q}") for q in range(QC)]
            tRs4 = [big.tile([128, RG, RPC], bf16, name=f"tR_s{q}", tag=f"tR_s{q}") for q in range(QR)]
            tRx4 = [big.tile([128, RG, RPC], bf16, name=f"tR_x{q}", tag=f"tR_x{q}") for q in range(QR)]
            tRz4 = [big.tile([128, RG, RPC], bf16, name=f"tR_z{q}", tag=f"tR_z{q}") for q in range(QR)]

            def tC(pieces, c):
                return pieces[c // 4][:, c % 4, :]

            def tR(pieces, a, c8):
                return pieces[c8][:, a, :]

            # cols of S/Z first (P0), then rows interleaved (B1), then cols of X (A2)
            for q in range(QC):
                nc.sync.dma_start(tCs4[q][:], sc_e[:, q * 4:(q + 1) * 4, :])
                nc.sync.dma_start(tCz4[q][:], zc_e[:, q * 4:(q + 1) * 4, :])
            for q in range(QR):
                nc.sync.dma_start(tRs4[q][:], sr_e[:, :, q * RPC:(q + 1) * RPC])
                nc.sync.dma_start(tRx4[q][:], xr_e[:, :, q * RPC:(q + 1) * RPC])
                nc.sync.dma_start(tRz4[q][:], zr_e[:, :, q * RPC:(q + 1) * RPC])
            for q in range(QC):
                nc.sync.dma_start(tCx4[q][:], xc_e[:, q * 4:(q + 1) * 4, :])

            ident = small.tile([128, 128], f32, tag="ident")
            make_identity(nc, ident[:])
            onesb = small.tile([128, 2], bf16, tag="onesb")
            nc.gpsimd.memset(onesb[:, 0:1], 1.0)
            nc.gpsimd.memset(onesb[:, 1:2], BETA)

            wsink = small.tile([1, 64], f32, tag="wsink")

            def warm_chain(n):
                """Chained dummy matmuls to keep the PE clock (HAM) warm while
                waiting on a collective. Each link gates the next through a
                ScalarE copy, spreading PE activity over ~n*0.7us."""
                for _ in range(n):
                    wp = pwarm.tile([1, RPC], f32, tag="warm")
                    nc.tensor.matmul(wp[:], onesb[:, 0:1], tCs4[0][:, 0, :],
                                     start=True, stop=True)
                    nc.scalar.copy(wsink[:], wp[0:1, 0:64])

            # ---------- P0: u1 = (rsS + b rsZ)/192 ----------
            ps_p0 = pp.tile([1, RPC], f32, tag="accA")
            for c in range(NC_CH):
                nc.tensor.matmul(ps_p0[:], onesb[:, 0:1], tC(tCs4, c),
                                 start=(c == 0), stop=False)
            for c in range(NC_CH):
                nc.tensor.matmul(ps_p0[:], onesb[:, 1:2], tC(tCz4, c),
                                 start=False, stop=(c == NC_CH - 1))
            u1sb = small.tile([1, RPC], f32, tag="u1sb")
            nc.vector.tensor_scalar_mul(u1sb[:], ps_p0[:], 2.0 / (3.0 * R))
            nc.sync.dma_start(out_u1[:], u1sb[:])

            # ---------- B1 stationaries: yS1=1/(R(u1+a)), yX1=a*yS1, yZ1=1/(R u1)
            y1S = small.tile([1, RPC], f32, tag="yS")
            y1X = small.tile([1, RPC], f32, tag="yX")
            y1Z = small.tile([1, RPC], f32, tag="yZ")
            t1 = small.tile([1, RPC], f32, tag="t1")
            nc.vector.tensor_scalar(t1[:], u1sb[:], ALPHA, float(R),
                                    mybir.AluOpType.add, mybir.AluOpType.mult)
            nc.vector.reciprocal(y1S[:], t1[:])
            nc.vector.tensor_scalar_mul(y1X[:], y1S[:], ALPHA)
            nc.vector.tensor_scalar_mul(t1[:], u1sb[:], float(R))
            nc.vector.reciprocal(y1Z[:], t1[:])

            def transpose_rows_to_cols(yvecs):
                """list of [1, 512] f32 -> psum [128, nvec*RG] grouped per vector."""
                ps_t = ptrans.tile([128, 96], f32, tag="pt")
                for v, yv in enumerate(yvecs):
                    for a in range(RG):
                        nc.tensor.transpose(
                            ps_t[:, v * RG + a: v * RG + a + 1],
                            yv[0:1, a * 128:(a + 1) * 128],
                            ident[0:1, 0:1],
                        )
                return ps_t

            ps_t1 = transpose_rows_to_cols([y1S, y1X, y1Z])
            statS1 = small.tile([128, 2, RG], bf16, tag="statS1")
            statX1 = small.tile([128, 2, RG], bf16, tag="statX1")
            statZ1 = small.tile([128, 2, RG], bf16, tag="statZ1")
            nc.gpsimd.memset(statS1[:, 1, :], 1.0)
            nc.gpsimd.memset(statX1[:, 1, :], ALPHA)
            nc.gpsimd.memset(statZ1[:, 1, :], 1.0)
            nc.vector.tensor_copy(statS1[:, 0, :], ps_t1[:, 0:RG])
            nc.vector.tensor_copy(statX1[:, 0, :], ps_t1[:, RG:2 * RG])
            nc.vector.tensor_copy(statZ1[:, 0, :], ps_t1[:, 2 * RG:3 * RG])

            # ---------- B1 (contract rows): h1pre,csSX | v1pre,csZ -> AR1 ----------
            ar1_in = dram.tile([4, NC_CH, 128], f32, tag="ar1_in")

            def b_pass(statS, statX, statZ, nv, ar_in_t):
                for c8 in range(NJ8):
                    ps_sx = pstream.tile([2, RPC], f32, tag="sx")
                    ps_z = pstream.tile([2, RPC], f32, tag="z")
                    for a in range(RG):
                        nc.tensor.matmul(ps_sx[0:nv, :], statS[:, :, a] if nv == 2 else statS[:, a:a + 1],
                                         tR(tRs4, a, c8), start=(a == 0), stop=False)
                    for a in range(RG):
                        nc.tensor.matmul(ps_sx[0:nv, :], statX[:, :, a] if nv == 2 else statX[:, a:a + 1],
                                         tR(tRx4, a, c8), start=False, stop=(a == RG - 1))
                    for a in range(RG):
                        nc.tensor.matmul(ps_z[0:nv, :], statZ[:, :, a] if nv == 2 else statZ[:, a:a + 1],
                                         tR(tRz4, a, c8), start=(a == 0), stop=(a == RG - 1))
                    st = stg.tile([34, RG, 128], f32, tag="st")
                    nc.scalar.copy(st[0:nv].rearrange("p a b -> p (a b)"), ps_sx[0:nv, :])
                    nc.vector.tensor_copy(st[32:32 + nv].rearrange("p a b -> p (a b)"), ps_z[0:nv, :])
                    nc.gpsimd.dma_start(ar_in_t[0:nv, c8 * RG:(c8 + 1) * RG, :], st[0:nv])
                    nc.gpsimd.dma_start(ar_in_t[nv:2 * nv, c8 * RG:(c8 + 1) * RG, :], st[32:32 + nv])

            b_pass(statS1, statX1, statZ1, 2, ar1_in)
            nc.gpsimd.collective_compute(
                "AllReduce", mybir.AluOpType.add, replica_groups=groups,
                ins=[ar1_in.opt()], outs=[ar1_out[:].opt()])
            warm_chain(16)

            # ---------- A2 stationaries from AR1 ----------
            h1 = small.tile([NC_CH, 128], f32, tag="h1")
            v1 = small.tile([NC_CH, 128], f32, tag="v1")
            nc.sync.dma_start(h1[:], ar1_out[0])
            nc.sync.dma_start(v1[:], ar1_out[2])
            vstk = small.tile([96, 128], f32, tag="vstk")
            tmp32 = small.tile([NC_CH, 128], f32, tag="tmp32")
            # invdS = 1/(R*(h1 + b*v1)); zhalf = b*invdS; invh1 = 1/(R*h1)
            nc.vector.tensor_scalar_mul(tmp32[:], v1[:], BETA)
            nc.vector.tensor_add(tmp32[:], tmp32[:], h1[:])
            nc.vector.tensor_scalar_mul(tmp32[:], tmp32[:], float(R))
            nc.vector.reciprocal(vstk[0:32, :], tmp32[:])
            nc.vector.tensor_scalar_mul(vstk[32:64, :], vstk[0:32, :], BETA)
            nc.vector.tensor_scalar_mul(tmp32[:], h1[:], float(R))
            nc.vector.reciprocal(vstk[64:96, :], tmp32[:])
            ps_t2 = ptrans.tile([128, 96], f32, tag="pt")
            nc.tensor.transpose(ps_t2[:], vstk[:], ident[0:96, 0:96])
            statA_sz = small.tile([128, 64], bf16, tag="statA_sz")
            statA_x = small.tile([128, 2, NC_CH], bf16, tag="statA_x")
            nc.vector.tensor_copy(statA_sz[:], ps_t2[:, 0:64])
            nc.gpsimd.memset(statA_x[:, 1, :], 1.0)
            nc.vector.tensor_copy(statA_x[:, 0, :], ps_t2[:, 64:96])

            # ---------- A2 (contract cols): u2 | w1, rsX ----------
            ps_a2u = pp.tile([1, RPC], f32, tag="accA")
            ps_a2wx = pp.tile([2, RPC], f32, tag="accB")
            for c in range(NC_CH):
                nc.tensor.matmul(ps_a2u[:], statA_sz[:, c:c + 1], tC(tCs4, c),
                                 start=(c == 0), stop=False)
            for c in range(NC_CH):
                nc.tensor.matmul(ps_a2u[:], statA_sz[:, 32 + c:33 + c], tC(tCz4, c),
                                 start=False, stop=(c == NC_CH - 1))
            for c in range(NC_CH):
                nc.tensor.matmul(ps_a2wx[:], statA_x[:, :, c], tC(tCx4, c),
                                 start=(c == 0), stop=(c == NC_CH - 1))
            a2u = small.tile([1, RPC], f32, tag="a2u")
            a2wx = small.tile([2, RPC], f32, tag="a2wx")
            nc.scalar.copy(a2u[:], ps_a2u[:])
            nc.scalar.copy(a2wx[:], ps_a2wx[:])
            nc.sync.dma_start(out_u2[:], a2u[:])
            nc.sync.dma_start(out_rsx[:], a2wx[1:2, :])
            warm_chain(6)

            # ---------- B2 stationaries: yS2=1/(R(u2+a w1)), yX2=a yS2, yZ2=1/(R u2)
            y2S = small.tile([1, RPC], f32, tag="yS")
            y2X = small.tile([1, RPC], f32, tag="yX")
            y2Z = small.tile([1, RPC], f32, tag="yZ")
            t2 = small.tile([1, RPC], f32, tag="t1")
            nc.vector.tensor_scalar_mul(t2[:], a2wx[0:1, :], ALPHA)
            nc.vector.tensor_add(t2[:], t2[:], a2u[:])
            nc.vector.tensor_scalar_mul(t2[:], t2[:], float(R))
            nc.vector.reciprocal(y2S[:], t2[:])
            nc.vector.tensor_scalar_mul(y2X[:], y2S[:], ALPHA)
            nc.vector.tensor_scalar_mul(t2[:], a2u[:], float(R))
            nc.vector.reciprocal(y2Z[:], t2[:])
            ps_t3 = transpose_rows_to_cols([y2S, y2X, y2Z])
            statS2 = small.tile([128, RG], bf16, tag="statS2")
            statX2 = small.tile([128, RG], bf16, tag="statX2")
            statZ2 = small.tile([128, RG], bf16, tag="statZ2")
            nc.vector.tensor_copy(statS2[:], ps_t3[:, 0:RG])
            nc.vector.tensor_copy(statX2[:], ps_t3[:, RG:2 * RG])
            nc.vector.tensor_copy(statZ2[:], ps_t3[:, 2 * RG:3 * RG])

            # ---------- B2 -> AR2 ----------
            ar2_in = dram.tile([2, NC_CH, 128], f32, tag="ar2_in")
            b_pass(statS2, statX2, statZ2, 1, ar2_in)
            nc.gpsimd.collective_compute(
                "AllReduce", mybir.AluOpType.add, replica_groups=groups,
                ins=[ar2_in.opt()], outs=[ar2_out[:].opt()])
            warm_chain(12)

            # ---------- A3: w2 = X (1/h2) / R ----------
            h2 = small.tile([NC_CH, 128], f32, tag="h1")
            nc.sync.dma_start(h2[:], ar2_out[0])
            nc.vector.tensor_scalar_mul(tmp32[:], h2[:], float(R))
            nc.vector.reciprocal(vstk[0:32, :], tmp32[:])
            ps_t4 = ptrans.tile([128, 96], f32, tag="pt")
            nc.tensor.transpose(ps_t4[:, 0:32], vstk[0:32, :], ident[0:32, 0:32])
            statA3 = small.tile([128, NC_CH], bf16, tag="statA3")
            nc.vector.tensor_copy(statA3[:], ps_t4[:, 0:32])
            ps_a3 = pp.tile([2, RPC], f32, tag="accB")
            for c in range(NC_CH):
                nc.tensor.matmul(ps_a3[0:1, :], statA3[:, c:c + 1], tC(tCx4, c),
                                 start=(c == 0), stop=(c == NC_CH - 1))
            w2sb = small.tile([1, RPC], f32, tag="u1sb")
            nc.scalar.copy(w2sb[:], ps_a3[0:1, :])
            nc.sync.dma_start(out_w2[:], w2sb[:])

            # ---------- replicated outputs (DRAM->DRAM) ----------
            nc.sync.dma_start(out_h2[:], ar2_out[0])
            nc.sync.dma_start(out_v2[:], ar2_out[1])
            nc.sync.dma_start(out_cssx[:], ar1_out[1])
            nc.sync.dma_start(out_csz[:], ar1_out[3])

    nc.compile()
    return nc


def _finale(res):
    """Assemble the scalar objective from per-core device outputs (float64)."""
    u1 = np.concatenate([np.asarray(res[i]["u1"], np.float64).ravel() for i in range(N_CORES)])
    u2 = np.concatenate([np.asarray(res[i]["u2"], np.float64).ravel() for i in range(N_CORES)])
    w2 = np.concatenate([np.asarray(res[i]["w2"], np.float64).ravel() for i in range(N_CORES)])
    rsx = np.concatenate([np.asarray(res[i]["rsx"], np.float64).ravel() for i in range(N_CORES)])
    h2 = np.asarray(res[0]["h2"], np.float64).ravel()
    v2 = np.asarray(res[0]["v2"], np.float64).ravel()
    cssx = np.asarray(res[0]["cssx"], np.float64).ravel()
    csz = np.asarray(res[0]["csz"], np.float64).ravel()

    rs_sz = u1 * (1.5 * R)          # = rsS + b*rsZ  (u1 = rs_sz*2/(3R))
    lR = np.log(R)
    term1 = R * (u2.sum() * h2.sum() + ALPHA * w2.sum() * h2.sum()
                 + BETA * u2.sum() * v2.sum())
    O = (term1
         - (rs_sz.sum() + ALPHA * rsx.sum()) * lR
         - (np.log(u2) * rs_sz).sum()
         - ALPHA * (np.log(w2) * rsx).sum()
         - (np.log(h2) * cssx).sum()
         - BETA * (np.log(v2) * csz).sum())
    return np.float32(O)


def _numpy_fallback(S, Z, X, U, H, W, V):
    """Faithful CPU implementation (only used if factors are not all-ones)."""
    S, Z, X, U, H, W, V = [np.asarray(a, np.float32) for a in (S, Z, X, U, H, W, V)]

    def obj(Sp, Xp, Zp):
        return ((Sp - S * np.log(Sp)).sum()
                + ALPHA * (Xp - X * np.log(Xp)).sum()
                + BETA * (Zp - Z * np.log(Zp)).sum())

    Sp = U @ H; Xp = W @ H; Zp = U @ V
    Sd = S / Sp; Xd = X / Xp; Zd = Z / Zp
    O = obj(Sp, Xp, Zp)
    for _ in range(2):
        dHV = H + BETA * V
        U = U * (Sd @ (H / dHV).T + Zd @ ((BETA * V) / dHV).T)
        Sp = U @ H; Zp = U @ V; Sd = S / Sp; Zd = Z / Zp
        dUW = U + ALPHA * W
        H = H * ((U / dUW).T @ Sd + ((ALPHA * W) / dUW).T @ Xd)
        Sp = U @ H; Xp = W @ H; Sd = S / Sp; Xd = X / Xp
        W = W * Xd.sum(axis=1, keepdims=True)
        Xp = W @ H; Xd = X / Xp
        V = V * Zd.sum(axis=0, keepdims=True)
        Zp = U @ V; Zd = Z / Zp
        O = obj(Sp, Xp, Zp)
    return np.float32(O)


def kernel(S, Z, X, U, H, W, V):
    if not (np.all(np.asarray(U) == 1) and np.all(np.asarray(H) == 1)
            and np.all(np.asarray(W) == 1) and np.all(np.asarray(V) == 1)):
        return _numpy_fallback(S, Z, X, U, H, W, V)

    from concourse.bass_utils import run_bass_kernel_spmd

    if "nc" not in _CACHED:
        _CACHED["nc"] = _build()
    nc = _CACHED["nc"]

    Sb = np.asarray(S, np.float32).astype(ml_dtypes.bfloat16)
    Xb = np.asarray(X, np.float32).astype(ml_dtypes.bfloat16)
    Zb = np.asarray(Z, np.float32).astype(ml_dtypes.bfloat16)

    def row_layout(shard):
        # [512, 4096] -> [128(p), RG(a), 4096(j)], per-partition contiguous
        return np.ascontiguousarray(shard.reshape(RG, 128, N).transpose(1, 0, 2))

    def col_layout(shard):
        # [512, 4096] -> [128(p), NC_CH(c), 512(l)] where (c,p) indexes column j
        return np.ascontiguousarray(
            shard.T.reshape(NC_CH, 128, RPC).transpose(1, 0, 2))

    in_maps = []
    for c in range(N_CORES):
        rows = slice(c * RPC, (c + 1) * RPC)
        sr_ = Sb[rows]; xr_ = Xb[rows]; zr_ = Zb[rows]
        in_maps.append({
            "sr": row_layout(sr_), "xr": row_layout(xr_), "zr": row_layout(zr_),
            "sc": col_layout(sr_), "xc": col_layout(xr_), "zc": col_layout(zr_),
        })

    res = run_bass_kernel_spmd(nc, in_maps, core_ids=list(range(N_CORES)))
    return _finale(res.results)


if __name__ == "__main__":
    import reference
    inputs = reference.setup_inputs()
    inputs = {k: np.asarray(v) for k, v in inputs.items()}
    print("kernel:", kernel(**inputs))


# revision 23
# speedup vs baseline: 1.1873x; 1.1873x over previous
"""Trainium2 8-core kernel for nn_ACCSLP_59485297050024.

The reference is a multiplicative-update NMF-style solver on N=4096 nodes with
rank R=128 and N_ITERS=2, returning a scalar objective O.

Because U, H, W, V are initialized to all-ones (per the problem's input spec),
every multiplicative update keeps each factor CONSTANT along the rank axis, so
the whole computation collapses exactly to rank-1 vector recurrences:

    u1 = (rowsum(S) + b*rowsum(Z)) * 2/(3R)
    h1 = (S + a*X)^T (1/e1) / R,  e1 = u1 + a       v1 = Z^T (1/u1) / R
    w1 = X (1/h1) / R,   u2 = (S + b*Z)(1/d1) / R,  d1 = h1 + b*v1
    h2 = (S + a*X)^T (1/e2) / R,  e2 = u2 + a*w1    v2 = Z^T (1/u2) / R
    w2 = X (1/h2) / R
    O  = R[Su2 Sh2 + a Sw2 Sh2 + b Su2 Sv2]
         - (sum(S) + a sum(X) + b sum(Z)) log R
         - <log u2, rsS + b rsZ> - a <log w2, rsX>
         - <log h2, csS + a csX> - b <log v2, csZ>

(verified exact vs the reference, rel err ~2e-16 in float64).

S only ever appears combined: P = S + a*X (h updates, colsums) and
Q = S + b*Z (u updates), so the device streams FOUR matrices (P, Z row-major;
Q, X col-major), not six.

Device strategy (8 NeuronCores): row-shard the matrices (512 rows/core); keep
row-major shards of P/Z and column-major (transposed) shards of Q/X resident
in SBUF (bf16). Every contraction is a TensorE matmul with a tiny stationary
vector:
  - "B" passes (contract over rows -> h/v/colsums) use row-major tiles;
    partials are AllReduce-summed across cores (2 AllReduces total).
  - "A" passes (contract over cols -> u/w/rowsums) use col-major tiles;
    results stay core-local (each core owns its 512 rows of u/w).
A dummy AllReduce issued first absorbs the per-execution collective warmup and
core start skew under the DMA load phase; chained dummy matmuls keep the PE
clock warm across the real AllReduce waits. The final scalar assembly (logs +
dot products on 4096-vectors) runs on host in float64.
"""

import numpy as np
import ml_dtypes

N = 4096
R = 128
ALPHA = 0.5
BETA = 0.5
N_CORES = 8
RPC = N // N_CORES          # rows per core = 512
RG = RPC // 128             # row groups per core = 4
NC_CH = N // 128            # 128-column chunks = 32
NJ8 = N // 512              # 512-column chunks = 8

_CACHED = {}


def _build():
    import concourse.mybir as mybir
    import concourse.tile as tile
    from concourse import bacc
    from concourse.masks import make_identity

    bf16 = mybir.dt.bfloat16
    f32 = mybir.dt.float32

    nc = bacc.Bacc("TRN2", target_bir_lowering=False, debug=False,
                   num_devices=N_CORES, dynamic_dma_scratch_size=8192)

    # per-core external I/O (host supplies per-partition-contiguous layouts)
    rp_e = nc.declare_dram_parameter("rp", [128, RG, N], bf16, isOutput=False)
    rz_e = nc.declare_dram_parameter("rz", [128, RG, N], bf16, isOutput=False)
    cq_e = nc.declare_dram_parameter("cq", [128, NC_CH, RPC], bf16, isOutput=False)
    cx_e = nc.declare_dram_parameter("cx", [128, NC_CH, RPC], bf16, isOutput=False)
    out_u1 = nc.declare_dram_parameter("u1", [1, RPC], f32, isOutput=True)
    out_u2 = nc.declare_dram_parameter("u2", [1, RPC], f32, isOutput=True)
    out_w2 = nc.declare_dram_parameter("w2", [1, RPC], f32, isOutput=True)
    out_rsx = nc.declare_dram_parameter("rsx", [1, RPC], f32, isOutput=True)
    out_h2 = nc.declare_dram_parameter("h2", [NC_CH, 128], f32, isOutput=True)
    out_v2 = nc.declare_dram_parameter("v2", [NC_CH, 128], f32, isOutput=True)
    out_cssx = nc.declare_dram_parameter("cssx", [NC_CH, 128], f32, isOutput=True)
    out_csz = nc.declare_dram_parameter("csz", [NC_CH, 128], f32, isOutput=True)

    ar1_out = nc.dram_tensor("ar1_out", [4, NC_CH, 128], f32, addr_space="Shared")
    ar2_out = nc.dram_tensor("ar2_out", [2, NC_CH, 128], f32, addr_space="Shared")
    groups = [list(range(N_CORES))]

    with tile.TileContext(nc) as tc:
        with (
            tc.tile_pool(name="big", bufs=1) as big,
            tc.tile_pool(name="small", bufs=1) as small,
            tc.tile_pool(name="stg", bufs=2) as stg,
            tc.tile_pool(name="pp", bufs=1, space="PSUM") as pp,
            tc.tile_pool(name="pstream", bufs=2, space="PSUM") as pstream,
            tc.tile_pool(name="ptrans", bufs=1, space="PSUM") as ptrans,
            tc.tile_pool(name="pwarm", bufs=1, space="PSUM") as pwarm,
            tc.tile_pool(name="dram", bufs=1, space="DRAM") as dram,
        ):
            # ---------- cross-core barrier first ----------
            # dummy AllReduce: absorbs per-execution collective warmup + core
            # start skew while the DMA loads run. Input is never written
            # (garbage) and output never read -- must not wait on anything.
            sync_in = nc.dram_tensor("sync_in", [1, 128], f32)
            sync_out = nc.dram_tensor("sync_out", [1, 128], f32, addr_space="Shared")
            with tc.high_priority():
                nc.gpsimd.collective_compute(
                    "AllReduce", mybir.AluOpType.add, replica_groups=groups,
                    ins=[sync_in[:].opt()], outs=[sync_out[:].opt()])

            # ---------- resident loads (pieces, ordered for pipelining) ----------
            QC = 4   # col-tile pieces per matrix (8 chunks each)
            QR = 4   # row-tile pieces per matrix (1024 cols each)
            W4 = N // QR
            tCq4 = [big.tile([128, 8, RPC], bf16, name=f"tC_q{q}", tag=f"tC_q{q}") for q in range(QC)]
            tCx4 = [big.tile([128, 8, RPC], bf16, name=f"tC_x{q}", tag=f"tC_x{q}") for q in range(QC)]
            tRp4 = [big.tile([128, RG, W4], bf16, name=f"tR_p{q}", tag=f"tR_p{q}") for q in range(QR)]
            tRz4 = [big.tile([128, RG, W4], bf16, name=f"tR_z{q}", tag=f"tR_z{q}") for q in range(QR)]

            def tC(pieces, c):
                return pieces[c // 8][:, c % 8, :]

            def tR(pieces, a, c8):
                w = (c8 % 2) * RPC
                return pieces[c8 // 2][:, a, w:w + RPC]

            # cols of Q first (P0), then rows interleaved (B1), then cols of X
            for q in range(QC):
                nc.sync.dma_start(tCq4[q][:], cq_e[:, q * 8:(q + 1) * 8, :])
            for q in range(QR):
                nc.sync.dma_start(tRp4[q][:], rp_e[:, :, q * W4:(q + 1) * W4])
                nc.sync.dma_start(tRz4[q][:], rz_e[:, :, q * W4:(q + 1) * W4])
            for q in range(QC):
                nc.sync.dma_start(tCx4[q][:], cx_e[:, q * 8:(q + 1) * 8, :])

            ident = small.tile([128, 128], f32, tag="ident")
            make_identity(nc, ident[:])
            onesb = small.tile([128, 2], bf16, tag="onesb")
            nc.gpsimd.memset(onesb[:, 0:1], 1.0)
            nc.gpsimd.memset(onesb[:, 1:2], BETA)

            wsink = small.tile([1, 64], f32, tag="wsink")

            def warm_chain(n):
                """Chained dummy matmuls to keep the PE clock (HAM) warm while
                waiting on a collective. Each link gates the next through a
                ScalarE copy, spreading PE activity over the wait."""
                for _ in range(n):
                    wp = pwarm.tile([1, RPC], f32, tag="warm")
                    nc.tensor.matmul(wp[:], onesb[:, 0:1], tCq4[0][:, 0, :],
                                     start=True, stop=True)
                    nc.scalar.copy(wsink[:], wp[0:1, 0:64])

            # ---------- P0: u1 = rowsum(Q) * 2/(3R) ----------
            ps_p0 = pp.tile([1, RPC], f32, tag="accA")
            for c in range(NC_CH):
                nc.tensor.matmul(ps_p0[:], onesb[:, 0:1], tC(tCq4, c),
                                 start=(c == 0), stop=(c == NC_CH - 1))
            u1sb = small.tile([1, RPC], f32, tag="u1sb")
            nc.vector.tensor_scalar_mul(u1sb[:], ps_p0[:], 2.0 / (3.0 * R))
            nc.sync.dma_start(out_u1[:], u1sb[:])

            # ---------- B1 stationaries: yP1 = 1/(R(u1+a)), yZ1 = 1/(R u1) ----
            y1P = small.tile([1, RPC], f32, tag="yP")
            y1Z = small.tile([1, RPC], f32, tag="yZ")
            t1 = small.tile([1, RPC], f32, tag="t1")
            nc.vector.tensor_scalar(t1[:], u1sb[:], ALPHA, float(R),
                                    mybir.AluOpType.add, mybir.AluOpType.mult)
            nc.vector.reciprocal(y1P[:], t1[:])
            nc.vector.tensor_scalar_mul(t1[:], u1sb[:], float(R))
            nc.vector.reciprocal(y1Z[:], t1[:])

            def transpose_rows_to_cols(yvecs):
                """list of [1, 512] f32 -> psum [128, nvec*RG] grouped per vector."""
                ps_t = ptrans.tile([128, 96], f32, tag="pt")
                for v, yv in enumerate(yvecs):
                    for a in range(RG):
                        nc.tensor.transpose(
                            ps_t[:, v * RG + a: v * RG + a + 1],
                            yv[0:1, a * 128:(a + 1) * 128],
                            ident[0:1, 0:1],
                        )
                return ps_t

            ps_t1 = transpose_rows_to_cols([y1P, y1Z])
            statP1 = small.tile([128, 2, RG], bf16, tag="statP1")
            statZ1 = small.tile([128, 2, RG], bf16, tag="statZ1")
            nc.gpsimd.memset(statP1[:, 1, :], 1.0)
            nc.gpsimd.memset(statZ1[:, 1, :], 1.0)
            nc.vector.tensor_copy(statP1[:, 0, :], ps_t1[:, 0:RG])
            nc.vector.tensor_copy(statZ1[:, 0, :], ps_t1[:, RG:2 * RG])

            # ---------- B passes (contract rows) ----------
            ar1_in = dram.tile([4, NC_CH, 128], f32, tag="ar1_in")

            def b_pass(statP, statZ, nv, ar_in_t):
                for c8 in range(NJ8):
                    ps_h = pstream.tile([2, RPC], f32, tag="sx")
                    ps_v = pstream.tile([2, RPC], f32, tag="z")
                    for a in range(RG):
                        nc.tensor.matmul(ps_h[0:nv, :],
                                         statP[:, :, a] if nv == 2 else statP[:, a:a + 1],
                                         tR(tRp4, a, c8), start=(a == 0), stop=(a == RG - 1))
                    for a in range(RG):
                        nc.tensor.matmul(ps_v[0:nv, :],
                                         statZ[:, :, a] if nv == 2 else statZ[:, a:a + 1],
                                         tR(tRz4, a, c8), start=(a == 0), stop=(a == RG - 1))
                    st = stg.tile([34, RG, 128], f32, tag="st")
                    nc.scalar.copy(st[0:nv].rearrange("p a b -> p (a b)"), ps_h[0:nv, :])
                    nc.vector.tensor_copy(st[32:32 + nv].rearrange("p a b -> p (a b)"), ps_v[0:nv, :])
                    nc.gpsimd.dma_start(ar_in_t[0:nv, c8 * RG:(c8 + 1) * RG, :], st[0:nv])
                    nc.gpsimd.dma_start(ar_in_t[nv:2 * nv, c8 * RG:(c8 + 1) * RG, :], st[32:32 + nv])

            b_pass(statP1, statZ1, 2, ar1_in)
            nc.gpsimd.collective_compute(
                "AllReduce", mybir.AluOpType.add, replica_groups=groups,
                ins=[ar1_in.opt()], outs=[ar1_out[:].opt()])
            warm_chain(16)

            # ---------- A2 stationaries from AR1 ----------
            # ar1 rows: 0 = h1pre, 1 = colsum(P), 2 = v1pre, 3 = colsum(Z)
            h1 = small.tile([NC_CH, 128], f32, tag="h1")
            v1 = small.tile([NC_CH, 128], f32, tag="v1")
            nc.sync.dma_start(h1[:], ar1_out[0])
            nc.sync.dma_start(v1[:], ar1_out[2])
            vstk = small.tile([64, 128], f32, tag="vstk")
            tmp32 = small.tile([NC_CH, 128], f32, tag="tmp32")
            # vstk[0:32] = 1/(R*(h1 + b*v1))   (Q stationary)
            # vstk[32:64] = 1/(R*h1)           (X stationary, with ones col)
            nc.vector.tensor_scalar_mul(tmp32[:], v1[:], BETA)
            nc.vector.tensor_add(tmp32[:], tmp32[:], h1[:])
            nc.vector.tensor_scalar_mul(tmp32[:], tmp32[:], float(R))
            nc.vector.reciprocal(vstk[0:32, :], tmp32[:])
            nc.vector.tensor_scalar_mul(tmp32[:], h1[:], float(R))
            nc.vector.reciprocal(vstk[32:64, :], tmp32[:])
            ps_t2 = ptrans.tile([128, 96], f32, tag="pt")
            nc.tensor.transpose(ps_t2[:, 0:64], vstk[:], ident[0:64, 0:64])
            statA_q = small.tile([128, NC_CH], bf16, tag="statA_q")
            statA_x = small.tile([128, 2, NC_CH], bf16, tag="statA_x")
            nc.vector.tensor_copy(statA_q[:], ps_t2[:, 0:NC_CH])
            nc.gpsimd.memset(statA_x[:, 1, :], 1.0)
            nc.vector.tensor_copy(statA_x[:, 0, :], ps_t2[:, NC_CH:2 * NC_CH])

            # ---------- A2 (contract cols): u2 | w1, rsX ----------
            ps_a2u = pp.tile([1, RPC], f32, tag="accA")
            ps_a2wx = pp.tile([2, RPC], f32, tag="accB")
            for c in range(NC_CH):
                nc.tensor.matmul(ps_a2u[:], statA_q[:, c:c + 1], tC(tCq4, c),
                                 start=(c == 0), stop=(c == NC_CH - 1))
            for c in range(NC_CH):
                nc.tensor.matmul(ps_a2wx[:], statA_x[:, :, c], tC(tCx4, c),
                                 start=(c == 0), stop=(c == NC_CH - 1))
            a2u = small.tile([1, RPC], f32, tag="a2u")
            a2wx = small.tile([2, RPC], f32, tag="a2wx")
            nc.scalar.copy(a2u[:], ps_a2u[:])
            nc.scalar.copy(a2wx[:], ps_a2wx[:])
            nc.sync.dma_start(out_u2[:], a2u[:])
            nc.sync.dma_start(out_rsx[:], a2wx[1:2, :])
            warm_chain(6)

            # ---------- B2 stationaries: yP2 = 1/(R(u2+a w1)), yZ2 = 1/(R u2)
            y2P = small.tile([1, RPC], f32, tag="yP")
            y2Z = small.tile([1, RPC], f32, tag="yZ")
            t2 = small.tile([1, RPC], f32, tag="t1")
            nc.vector.tensor_scalar_mul(t2[:], a2wx[0:1, :], ALPHA)
            nc.vector.tensor_add(t2[:], t2[:], a2u[:])
            nc.vector.tensor_scalar_mul(t2[:], t2[:], float(R))
            nc.vector.reciprocal(y2P[:], t2[:])
            nc.vector.tensor_scalar_mul(t2[:], a2u[:], float(R))
            nc.vector.reciprocal(y2Z[:], t2[:])
            ps_t3 = transpose_rows_to_cols([y2P, y2Z])
            statP2 = small.tile([128, RG], bf16, tag="statP2")
            statZ2 = small.tile([128, RG], bf16, tag="statZ2")
            nc.vector.tensor_copy(statP2[:], ps_t3[:, 0:RG])
            nc.vector.tensor_copy(statZ2[:], ps_t3[:, RG:2 * RG])

            # ---------- B2 -> AR2 (rows: h2pre, v2pre) ----------
            ar2_in = dram.tile([2, NC_CH, 128], f32, tag="ar2_in")
            b_pass(statP2, statZ2, 1, ar2_in)
            nc.gpsimd.collective_compute(
                "AllReduce", mybir.AluOpType.add, replica_groups=groups,
                ins=[ar2_in.opt()], outs=[ar2_out[:].opt()])
            warm_chain(12)

            # ---------- A3: w2 = X (1/h2) / R ----------
            h2 = small.tile([NC_CH, 128], f32, tag="h1")
            nc.sync.dma_start(h2[:], ar2_out[0])
            nc.vector.tensor_scalar_mul(tmp32[:], h2[:], float(R))
            nc.vector.reciprocal(vstk[0:32, :], tmp32[:])
            ps_t4 = ptrans.tile([128, 96], f32, tag="pt")
            nc.tensor.transpose(ps_t4[:, 0:32], vstk[0:32, :], ident[0:32, 0:32])
            statA3 = small.tile([128, NC_CH], bf16, tag="statA3")
            nc.vector.tensor_copy(statA3[:], ps_t4[:, 0:32])
            ps_a3 = pp.tile([2, RPC], f32, tag="accB")
            for c in range(NC_CH):
                nc.tensor.matmul(ps_a3[0:1, :], statA3[:, c:c + 1], tC(tCx4, c),
                                 start=(c == 0), stop=(c == NC_CH - 1))
            w2sb = small.tile([1, RPC], f32, tag="u1sb")
            nc.scalar.copy(w2sb[:], ps_a3[0:1, :])
            nc.sync.dma_start(out_w2[:], w2sb[:])

            # ---------- replicated outputs (DRAM->DRAM) ----------
            nc.sync.dma_start(out_h2[:], ar2_out[0])
            nc.sync.dma_start(out_v2[:], ar2_out[1])
            nc.sync.dma_start(out_cssx[:], ar1_out[1])
            nc.sync.dma_start(out_csz[:], ar1_out[3])

    nc.compile()
    return nc


def _finale(res):
    """Assemble the scalar objective from per-core device outputs (float64)."""
    u1 = np.concatenate([np.asarray(res[i]["u1"], np.float64).ravel() for i in range(N_CORES)])
    u2 = np.concatenate([np.asarray(res[i]["u2"], np.float64).ravel() for i in range(N_CORES)])
    w2 = np.concatenate([np.asarray(res[i]["w2"], np.float64).ravel() for i in range(N_CORES)])
    rsx = np.concatenate([np.asarray(res[i]["rsx"], np.float64).ravel() for i in range(N_CORES)])
    h2 = np.asarray(res[0]["h2"], np.float64).ravel()
    v2 = np.asarray(res[0]["v2"], np.float64).ravel()
    cssx = np.asarray(res[0]["cssx"], np.float64).ravel()
    csz = np.asarray(res[0]["csz"], np.float64).ravel()

    rs_sz = u1 * (1.5 * R)          # = rsS + b*rsZ  (u1 = rs_sz*2/(3R))
    lR = np.log(R)
    term1 = R * (u2.sum() * h2.sum() + ALPHA * w2.sum() * h2.sum()
                 + BETA * u2.sum() * v2.sum())
    # sum(S) + a*sum(X) + b*sum(Z) = sum(cssx) + b*sum(csz)
    O = (term1
         - (cssx.sum() + BETA * csz.sum()) * lR
         - (np.log(u2) * rs_sz).sum()
         - ALPHA * (np.log(w2) * rsx).sum()
         - (np.log(h2) * cssx).sum()
         - BETA * (np.log(v2) * csz).sum())
    return np.float32(O)


def _numpy_fallback(S, Z, X, U, H, W, V):
    """Faithful CPU implementation (only used if factors are not all-ones)."""
    S, Z, X, U, H, W, V = [np.asarray(a, np.float32) for a in (S, Z, X, U, H, W, V)]

    def obj(Sp, Xp, Zp):
        return ((Sp - S * np.log(Sp)).sum()
                + ALPHA * (Xp - X * np.log(Xp)).sum()
                + BETA * (Zp - Z * np.log(Zp)).sum())

    Sp = U @ H; Xp = W @ H; Zp = U @ V
    Sd = S / Sp; Xd = X / Xp; Zd = Z / Zp
    O = obj(Sp, Xp, Zp)
    for _ in range(2):
        dHV = H + BETA * V
        U = U * (Sd @ (H / dHV).T + Zd @ ((BETA * V) / dHV).T)
        Sp = U @ H; Zp = U @ V; Sd = S / Sp; Zd = Z / Zp
        dUW = U + ALPHA * W
        H = H * ((U / dUW).T @ Sd + ((ALPHA * W) / dUW).T @ Xd)
        Sp = U @ H; Xp = W @ H; Sd = S / Sp; Xd = X / Xp
        W = W * Xd.sum(axis=1, keepdims=True)
        Xp = W @ H; Xd = X / Xp
        V = V * Zd.sum(axis=0, keepdims=True)
        Zp = U @ V; Zd = Z / Zp
        O = obj(Sp, Xp, Zp)
    return np.float32(O)


def _make_in_maps(S, Z, X):
    S = np.asarray(S, np.float32)
    Z = np.asarray(Z, np.float32)
    X = np.asarray(X, np.float32)
    P = (S + ALPHA * X).astype(ml_dtypes.bfloat16)   # h-side combined matrix
    Q = (S + BETA * Z).astype(ml_dtypes.bfloat16)    # u-side combined matrix
    Xb = X.astype(ml_dtypes.bfloat16)
    Zb = Z.astype(ml_dtypes.bfloat16)

    def row_layout(shard):
        # [512, 4096] -> [128(p), RG(a), 4096(j)], per-partition contiguous
        return np.ascontiguousarray(shard.reshape(RG, 128, N).transpose(1, 0, 2))

    def col_layout(shard):
        # [512, 4096] -> [128(p), NC_CH(c), 512(l)] where (c,p) indexes column j
        return np.ascontiguousarray(
            shard.T.reshape(NC_CH, 128, RPC).transpose(1, 0, 2))

    in_maps = []
    for c in range(N_CORES):
        rows = slice(c * RPC, (c + 1) * RPC)
        in_maps.append({
            "rp": row_layout(P[rows]), "rz": row_layout(Zb[rows]),
            "cq": col_layout(Q[rows]), "cx": col_layout(Xb[rows]),
        })
    return in_maps


def kernel(S, Z, X, U, H, W, V):
    if not (np.all(np.asarray(U) == 1) and np.all(np.asarray(H) == 1)
            and np.all(np.asarray(W) == 1) and np.all(np.asarray(V) == 1)):
        return _numpy_fallback(S, Z, X, U, H, W, V)

    from concourse.bass_utils import run_bass_kernel_spmd

    if "nc" not in _CACHED:
        _CACHED["nc"] = _build()
    nc = _CACHED["nc"]

    in_maps = _make_in_maps(S, Z, X)
    res = run_bass_kernel_spmd(nc, in_maps, core_ids=list(range(N_CORES)))
    return _finale(res.results)


if __name__ == "__main__":
    import reference
    inputs = reference.setup_inputs()
    inputs = {k: np.asarray(v) for k, v in inputs.items()}
    print("kernel:", kernel(**inputs))


# revision 24
# speedup vs baseline: 1.1979x; 1.0089x over previous
"""Trainium2 8-core kernel for nn_ACCSLP_59485297050024.

The reference is a multiplicative-update NMF-style solver on N=4096 nodes with
rank R=128 and N_ITERS=2, returning a scalar objective O.

Because U, H, W, V are initialized to all-ones (per the problem's input spec),
every multiplicative update keeps each factor CONSTANT along the rank axis, so
the whole computation collapses exactly to rank-1 vector recurrences:

    u1 = (rowsum(S) + b*rowsum(Z)) * 2/(3R)
    h1 = (S + a*X)^T (1/e1) / R,  e1 = u1 + a       v1 = Z^T (1/u1) / R
    w1 = X (1/h1) / R,   u2 = (S + b*Z)(1/d1) / R,  d1 = h1 + b*v1
    h2 = (S + a*X)^T (1/e2) / R,  e2 = u2 + a*w1    v2 = Z^T (1/u2) / R
    w2 = X (1/h2) / R
    O  = R[Su2 Sh2 + a Sw2 Sh2 + b Su2 Sv2]
         - (sum(S) + a sum(X) + b sum(Z)) log R
         - <log u2, rsS + b rsZ> - a <log w2, rsX>
         - <log h2, csS + a csX> - b <log v2, csZ>

(verified exact vs the reference, rel err ~2e-16 in float64).

S only ever appears combined: P = S + a*X (h updates, colsums) and
Q = S + b*Z (u updates), so the device streams FOUR matrices (P, Z row-major;
Q, X col-major), not six.

Device strategy (8 NeuronCores): row-shard the matrices (512 rows/core); keep
row-major shards of P/Z and column-major (transposed) shards of Q/X resident
in SBUF (bf16). Every contraction is a TensorE matmul with a tiny stationary
vector:
  - "B" passes (contract over rows -> h/v/colsums) use row-major tiles;
    partials are AllReduce-summed across cores (2 AllReduces total).
  - "A" passes (contract over cols -> u/w/rowsums) use col-major tiles;
    results stay core-local (each core owns its 512 rows of u/w).
A dummy AllReduce issued first absorbs the per-execution collective warmup and
core start skew under the DMA load phase; chained dummy matmuls keep the PE
clock warm across the real AllReduce waits. The final scalar assembly (logs +
dot products on 4096-vectors) runs on host in float64.
"""

import numpy as np
import ml_dtypes

N = 4096
R = 128
ALPHA = 0.5
BETA = 0.5
N_CORES = 8
RPC = N // N_CORES          # rows per core = 512
RG = RPC // 128             # row groups per core = 4
NC_CH = N // 128            # 128-column chunks = 32
NJ8 = N // 512              # 512-column chunks = 8

_CACHED = {}


def _build():
    import concourse.mybir as mybir
    import concourse.tile as tile
    from concourse import bacc
    from concourse.masks import make_identity

    bf16 = mybir.dt.bfloat16
    f32 = mybir.dt.float32

    nc = bacc.Bacc("TRN2", target_bir_lowering=False, debug=False,
                   num_devices=N_CORES, dynamic_dma_scratch_size=8192)

    # per-core external I/O (host supplies per-partition-contiguous layouts)
    rp_e = nc.declare_dram_parameter("rp", [128, RG, N], bf16, isOutput=False)
    rz_e = nc.declare_dram_parameter("rz", [128, RG, N], bf16, isOutput=False)
    cq_e = nc.declare_dram_parameter("cq", [128, NC_CH, RPC], bf16, isOutput=False)
    cx_e = nc.declare_dram_parameter("cx", [128, NC_CH, RPC], bf16, isOutput=False)
    out_u1 = nc.declare_dram_parameter("u1", [1, RPC], f32, isOutput=True)
    out_u2 = nc.declare_dram_parameter("u2", [1, RPC], f32, isOutput=True)
    out_w2 = nc.declare_dram_parameter("w2", [1, RPC], f32, isOutput=True)
    out_rsx = nc.declare_dram_parameter("rsx", [1, RPC], f32, isOutput=True)
    out_h2 = nc.declare_dram_parameter("h2", [NC_CH, 128], f32, isOutput=True)
    out_v2 = nc.declare_dram_parameter("v2", [NC_CH, 128], f32, isOutput=True)
    out_cssx = nc.declare_dram_parameter("cssx", [NC_CH, 128], f32, isOutput=True)
    out_csz = nc.declare_dram_parameter("csz", [NC_CH, 128], f32, isOutput=True)

    ar1_out = nc.dram_tensor("ar1_out", [4, NC_CH, 128], f32, addr_space="Shared")
    ar2_out = nc.dram_tensor("ar2_out", [2, NC_CH, 128], f32, addr_space="Shared")
    groups = [list(range(N_CORES))]

    with tile.TileContext(nc) as tc:
        with (
            tc.tile_pool(name="big", bufs=1) as big,
            tc.tile_pool(name="small", bufs=1) as small,
            tc.tile_pool(name="stg", bufs=2) as stg,
            tc.tile_pool(name="pp", bufs=1, space="PSUM") as pp,
            tc.tile_pool(name="pstream", bufs=2, space="PSUM") as pstream,
            tc.tile_pool(name="ptrans", bufs=1, space="PSUM") as ptrans,
            tc.tile_pool(name="pwarm", bufs=1, space="PSUM") as pwarm,
            tc.tile_pool(name="dram", bufs=1, space="DRAM") as dram,
        ):
            # ---------- cross-core barrier first ----------
            # dummy AllReduce: absorbs per-execution collective warmup + core
            # start skew while the DMA loads run. Input is never written
            # (garbage) and output never read -- must not wait on anything.
            sync_in = nc.dram_tensor("sync_in", [1, 128], f32)
            sync_out = nc.dram_tensor("sync_out", [1, 128], f32, addr_space="Shared")
            with tc.high_priority():
                nc.gpsimd.collective_compute(
                    "AllReduce", mybir.AluOpType.add, replica_groups=groups,
                    ins=[sync_in[:].opt()], outs=[sync_out[:].opt()])

            # ---------- resident loads (pieces, ordered for pipelining) ----------
            QC = 4   # col-tile pieces per matrix (8 chunks each)
            QR = 4   # row-tile pieces per matrix (1024 cols each)
            W4 = N // QR
            tCq4 = [big.tile([128, 8, RPC], bf16, name=f"tC_q{q}", tag=f"tC_q{q}") for q in range(QC)]
            tCx4 = [big.tile([128, 8, RPC], bf16, name=f"tC_x{q}", tag=f"tC_x{q}") for q in range(QC)]
            tRp4 = [big.tile([128, RG, W4], bf16, name=f"tR_p{q}", tag=f"tR_p{q}") for q in range(QR)]
            tRz4 = [big.tile([128, RG, W4], bf16, name=f"tR_z{q}", tag=f"tR_z{q}") for q in range(QR)]

            def tC(pieces, c):
                return pieces[c // 8][:, c % 8, :]

            def tR(pieces, a, c8):
                w = (c8 % 2) * RPC
                return pieces[c8 // 2][:, a, w:w + RPC]

            # cols of Q first (P0), then rows interleaved (B1), then cols of X
            for q in range(QC):
                nc.sync.dma_start(tCq4[q][:], cq_e[:, q * 8:(q + 1) * 8, :])
            for q in range(QR):
                nc.sync.dma_start(tRp4[q][:], rp_e[:, :, q * W4:(q + 1) * W4])
                nc.sync.dma_start(tRz4[q][:], rz_e[:, :, q * W4:(q + 1) * W4])
            for q in range(QC):
                nc.sync.dma_start(tCx4[q][:], cx_e[:, q * 8:(q + 1) * 8, :])

            ident = small.tile([128, 128], f32, tag="ident")
            make_identity(nc, ident[:])
            onesb = small.tile([128, 2], bf16, tag="onesb")
            nc.gpsimd.memset(onesb[:, 0:1], 1.0)
            nc.gpsimd.memset(onesb[:, 1:2], BETA)

            wsink = small.tile([1, 64], f32, tag="wsink")

            def warm_chain(n):
                """Chained dummy matmuls to keep the PE clock (HAM) warm while
                waiting on a collective. Each link gates the next through a
                ScalarE copy, spreading PE activity over the wait."""
                for _ in range(n):
                    wp = pwarm.tile([1, RPC], f32, tag="warm")
                    nc.tensor.matmul(wp[:], onesb[:, 0:1], tCq4[0][:, 0, :],
                                     start=True, stop=True)
                    nc.scalar.copy(wsink[:], wp[0:1, 0:64])

            # ---------- P0: u1 = rowsum(Q) * 2/(3R) ----------
            ps_p0 = pp.tile([1, RPC], f32, tag="accA")
            for c in range(NC_CH):
                nc.tensor.matmul(ps_p0[:], onesb[:, 0:1], tC(tCq4, c),
                                 start=(c == 0), stop=(c == NC_CH - 1))
            u1sb = small.tile([1, RPC], f32, tag="u1sb")
            nc.vector.tensor_scalar_mul(u1sb[:], ps_p0[:], 2.0 / (3.0 * R))
            nc.sync.dma_start(out_u1[:], u1sb[:])

            # ---------- B1 stationaries: yP1 = 1/(R(u1+a)), yZ1 = 1/(R u1) ----
            y1P = small.tile([1, RPC], f32, tag="yP")
            y1Z = small.tile([1, RPC], f32, tag="yZ")
            t1 = small.tile([1, RPC], f32, tag="t1")
            nc.vector.tensor_scalar(t1[:], u1sb[:], ALPHA, float(R),
                                    mybir.AluOpType.add, mybir.AluOpType.mult)
            nc.vector.reciprocal(y1P[:], t1[:])
            nc.vector.reciprocal(y1Z[:], u1sb[:])   # 1/u1; 1/R folded in copy

            def transpose_rows_to_cols(yvecs):
                """list of [1, 512] f32 -> psum [128, nvec*RG] grouped per vector."""
                ps_t = ptrans.tile([128, 96], f32, tag="pt")
                for v, yv in enumerate(yvecs):
                    for a in range(RG):
                        nc.tensor.transpose(
                            ps_t[:, v * RG + a: v * RG + a + 1],
                            yv[0:1, a * 128:(a + 1) * 128],
                            ident[0:1, 0:1],
                        )
                return ps_t

            ps_t1 = transpose_rows_to_cols([y1P, y1Z])
            statP1 = small.tile([128, 2, RG], bf16, tag="statP1")
            statZ1 = small.tile([128, 2, RG], bf16, tag="statZ1")
            nc.gpsimd.memset(statP1[:, 1, :], 1.0)
            nc.gpsimd.memset(statZ1[:, 1, :], 1.0)
            nc.vector.tensor_copy(statP1[:, 0, :], ps_t1[:, 0:RG])
            nc.vector.tensor_scalar_mul(statZ1[:, 0, :], ps_t1[:, RG:2 * RG], 1.0 / R)

            # ---------- B passes (contract rows) ----------
            ar1_in = dram.tile([4, NC_CH, 128], f32, tag="ar1_in")

            def b_pass(statP, statZ, nv, ar_in_t):
                for c8 in range(NJ8):
                    ps_h = pstream.tile([2, RPC], f32, tag="sx")
                    ps_v = pstream.tile([2, RPC], f32, tag="z")
                    for a in range(RG):
                        nc.tensor.matmul(ps_h[0:nv, :],
                                         statP[:, :, a] if nv == 2 else statP[:, a:a + 1],
                                         tR(tRp4, a, c8), start=(a == 0), stop=(a == RG - 1))
                    for a in range(RG):
                        nc.tensor.matmul(ps_v[0:nv, :],
                                         statZ[:, :, a] if nv == 2 else statZ[:, a:a + 1],
                                         tR(tRz4, a, c8), start=(a == 0), stop=(a == RG - 1))
                    st = stg.tile([34, RG, 128], f32, tag="st")
                    nc.scalar.copy(st[0:nv].rearrange("p a b -> p (a b)"), ps_h[0:nv, :])
                    nc.vector.tensor_copy(st[32:32 + nv].rearrange("p a b -> p (a b)"), ps_v[0:nv, :])
                    nc.gpsimd.dma_start(ar_in_t[0:nv, c8 * RG:(c8 + 1) * RG, :], st[0:nv])
                    nc.gpsimd.dma_start(ar_in_t[nv:2 * nv, c8 * RG:(c8 + 1) * RG, :], st[32:32 + nv])

            b_pass(statP1, statZ1, 2, ar1_in)
            nc.gpsimd.collective_compute(
                "AllReduce", mybir.AluOpType.add, replica_groups=groups,
                ins=[ar1_in.opt()], outs=[ar1_out[:].opt()])
            warm_chain(20)
            nc.sync.dma_start(out_cssx[:], ar1_out[1])
            nc.sync.dma_start(out_csz[:], ar1_out[3])

            # ---------- A2 stationaries from AR1 ----------
            # ar1 rows: 0 = h1pre, 1 = colsum(P), 2 = v1pre, 3 = colsum(Z)
            h1 = small.tile([NC_CH, 128], f32, tag="h1")
            v1 = small.tile([NC_CH, 128], f32, tag="v1")
            nc.sync.dma_start(h1[:], ar1_out[0])
            nc.sync.dma_start(v1[:], ar1_out[2])
            vstk = small.tile([64, 128], f32, tag="vstk")
            tmp32 = small.tile([NC_CH, 128], f32, tag="tmp32")
            # vstk[0:32] = 1/(h1 + b*v1)  (Q stationary; 1/R folded in copy)
            # vstk[32:64] = 1/h1           (X stationary, with raw ones col)
            nc.vector.scalar_tensor_tensor(tmp32[:], v1[:], BETA, h1[:],
                                           mybir.AluOpType.mult, mybir.AluOpType.add)
            nc.vector.reciprocal(vstk[0:32, :], tmp32[:])
            nc.vector.reciprocal(vstk[32:64, :], h1[:])
            ps_t2 = ptrans.tile([128, 96], f32, tag="pt")
            nc.tensor.transpose(ps_t2[:, 0:64], vstk[:], ident[0:64, 0:64])
            statA_q = small.tile([128, NC_CH], bf16, tag="statA_q")
            statA_x = small.tile([128, 2, NC_CH], bf16, tag="statA_x")
            nc.vector.tensor_scalar_mul(statA_q[:], ps_t2[:, 0:NC_CH], 1.0 / R)
            nc.gpsimd.memset(statA_x[:, 1, :], 1.0)
            nc.vector.tensor_scalar_mul(statA_x[:, 0, :], ps_t2[:, NC_CH:2 * NC_CH], 1.0 / R)

            # ---------- A2 (contract cols): u2 | w1, rsX ----------
            ps_a2u = pp.tile([1, RPC], f32, tag="accA")
            ps_a2wx = pp.tile([2, RPC], f32, tag="accB")
            for c in range(NC_CH):
                nc.tensor.matmul(ps_a2u[:], statA_q[:, c:c + 1], tC(tCq4, c),
                                 start=(c == 0), stop=(c == NC_CH - 1))
            for c in range(NC_CH):
                nc.tensor.matmul(ps_a2wx[:], statA_x[:, :, c], tC(tCx4, c),
                                 start=(c == 0), stop=(c == NC_CH - 1))
            a2u = small.tile([1, RPC], f32, tag="a2u")
            a2wx = small.tile([2, RPC], f32, tag="a2wx")
            nc.scalar.copy(a2u[:], ps_a2u[:])
            nc.scalar.copy(a2wx[:], ps_a2wx[:])
            nc.sync.dma_start(out_u2[:], a2u[:])
            nc.sync.dma_start(out_rsx[:], a2wx[1:2, :])

            # ---------- B2 stationaries: yP2 = 1/(R(u2+a w1)), yZ2 = 1/(R u2)
            y2P = small.tile([1, RPC], f32, tag="yP")
            y2Z = small.tile([1, RPC], f32, tag="yZ")
            t2 = small.tile([1, RPC], f32, tag="t1")
            nc.vector.scalar_tensor_tensor(t2[:], a2wx[0:1, :], ALPHA, a2u[:],
                                           mybir.AluOpType.mult, mybir.AluOpType.add)
            nc.vector.reciprocal(y2P[:], t2[:])
            nc.vector.reciprocal(y2Z[:], a2u[:])
            ps_t3 = transpose_rows_to_cols([y2P, y2Z])
            statP2 = small.tile([128, RG], bf16, tag="statP2")
            statZ2 = small.tile([128, RG], bf16, tag="statZ2")
            nc.vector.tensor_scalar_mul(statP2[:], ps_t3[:, 0:RG], 1.0 / R)
            nc.vector.tensor_scalar_mul(statZ2[:], ps_t3[:, RG:2 * RG], 1.0 / R)

            # ---------- B2 -> AR2 (rows: h2pre, v2pre) ----------
            ar2_in = dram.tile([2, NC_CH, 128], f32, tag="ar2_in")
            b_pass(statP2, statZ2, 1, ar2_in)
            nc.gpsimd.collective_compute(
                "AllReduce", mybir.AluOpType.add, replica_groups=groups,
                ins=[ar2_in.opt()], outs=[ar2_out[:].opt()])

            # ---------- A3: w2 = X (1/h2) / R ----------
            h2 = small.tile([NC_CH, 128], f32, tag="h1")
            nc.sync.dma_start(h2[:], ar2_out[0])
            nc.vector.reciprocal(vstk[0:32, :], h2[:])
            ps_t4 = ptrans.tile([128, 96], f32, tag="pt")
            nc.tensor.transpose(ps_t4[:, 0:32], vstk[0:32, :], ident[0:32, 0:32])
            statA3 = small.tile([128, NC_CH], bf16, tag="statA3")
            nc.vector.tensor_scalar_mul(statA3[:], ps_t4[:, 0:32], 1.0 / R)
            ps_a3 = pp.tile([2, RPC], f32, tag="accB")
            for c in range(NC_CH):
                nc.tensor.matmul(ps_a3[0:1, :], statA3[:, c:c + 1], tC(tCx4, c),
                                 start=(c == 0), stop=(c == NC_CH - 1))
            w2sb = small.tile([1, RPC], f32, tag="u1sb")
            nc.scalar.copy(w2sb[:], ps_a3[0:1, :])
            nc.sync.dma_start(out_w2[:], w2sb[:])

            # ---------- replicated outputs (DRAM->DRAM) ----------
            nc.sync.dma_start(out_h2[:], ar2_out[0])
            nc.sync.dma_start(out_v2[:], ar2_out[1])

    nc.compile()
    return nc


def _finale(res):
    """Assemble the scalar objective from per-core device outputs (float64)."""
    u1 = np.concatenate([np.asarray(res[i]["u1"], np.float64).ravel() for i in range(N_CORES)])
    u2 = np.concatenate([np.asarray(res[i]["u2"], np.float64).ravel() for i in range(N_CORES)])
    w2 = np.concatenate([np.asarray(res[i]["w2"], np.float64).ravel() for i in range(N_CORES)])
    rsx = np.concatenate([np.asarray(res[i]["rsx"], np.float64).ravel() for i in range(N_CORES)])
    h2 = np.asarray(res[0]["h2"], np.float64).ravel()
    v2 = np.asarray(res[0]["v2"], np.float64).ravel()
    cssx = np.asarray(res[0]["cssx"], np.float64).ravel()
    csz = np.asarray(res[0]["csz"], np.float64).ravel()

    rs_sz = u1 * (1.5 * R)          # = rsS + b*rsZ  (u1 = rs_sz*2/(3R))
    lR = np.log(R)
    term1 = R * (u2.sum() * h2.sum() + ALPHA * w2.sum() * h2.sum()
                 + BETA * u2.sum() * v2.sum())
    # sum(S) + a*sum(X) + b*sum(Z) = sum(cssx) + b*sum(csz)
    O = (term1
         - (cssx.sum() + BETA * csz.sum()) * lR
         - (np.log(u2) * rs_sz).sum()
         - ALPHA * (np.log(w2) * rsx).sum()
         - (np.log(h2) * cssx).sum()
         - BETA * (np.log(v2) * csz).sum())
    return np.float32(O)


def _numpy_fallback(S, Z, X, U, H, W, V):
    """Faithful CPU implementation (only used if factors are not all-ones)."""
    S, Z, X, U, H, W, V = [np.asarray(a, np.float32) for a in (S, Z, X, U, H, W, V)]

    def obj(Sp, Xp, Zp):
        return ((Sp - S * np.log(Sp)).sum()
                + ALPHA * (Xp - X * np.log(Xp)).sum()
                + BETA * (Zp - Z * np.log(Zp)).sum())

    Sp = U @ H; Xp = W @ H; Zp = U @ V
    Sd = S / Sp; Xd = X / Xp; Zd = Z / Zp
    O = obj(Sp, Xp, Zp)
    for _ in range(2):
        dHV = H + BETA * V
        U = U * (Sd @ (H / dHV).T + Zd @ ((BETA * V) / dHV).T)
        Sp = U @ H; Zp = U @ V; Sd = S / Sp; Zd = Z / Zp
        dUW = U + ALPHA * W
        H = H * ((U / dUW).T @ Sd + ((ALPHA * W) / dUW).T @ Xd)
        Sp = U @ H; Xp = W @ H; Sd = S / Sp; Xd = X / Xp
        W = W * Xd.sum(axis=1, keepdims=True)
        Xp = W @ H; Xd = X / Xp
        V = V * Zd.sum(axis=0, keepdims=True)
        Zp = U @ V; Zd = Z / Zp
        O = obj(Sp, Xp, Zp)
    return np.float32(O)


def _make_in_maps(S, Z, X):
    S = np.asarray(S, np.float32)
    Z = np.asarray(Z, np.float32)
    X = np.asarray(X, np.float32)
    P = (S + ALPHA * X).astype(ml_dtypes.bfloat16)   # h-side combined matrix
    Q = (S + BETA * Z).astype(ml_dtypes.bfloat16)    # u-side combined matrix
    Xb = X.astype(ml_dtypes.bfloat16)
    Zb = Z.astype(ml_dtypes.bfloat16)

    def row_layout(shard):
        # [512, 4096] -> [128(p), RG(a), 4096(j)], per-partition contiguous
        return np.ascontiguousarray(shard.reshape(RG, 128, N).transpose(1, 0, 2))

    def col_layout(shard):
        # [512, 4096] -> [128(p), NC_CH(c), 512(l)] where (c,p) indexes column j
        return np.ascontiguousarray(
            shard.T.reshape(NC_CH, 128, RPC).transpose(1, 0, 2))

    in_maps = []
    for c in range(N_CORES):
        rows = slice(c * RPC, (c + 1) * RPC)
        in_maps.append({
            "rp": row_layout(P[rows]), "rz": row_layout(Zb[rows]),
            "cq": col_layout(Q[rows]), "cx": col_layout(Xb[rows]),
        })
    return in_maps


def kernel(S, Z, X, U, H, W, V):
    if not (np.all(np.asarray(U) == 1) and np.all(np.asarray(H) == 1)
            and np.all(np.asarray(W) == 1) and np.all(np.asarray(V) == 1)):
        return _numpy_fallback(S, Z, X, U, H, W, V)

    from concourse.bass_utils import run_bass_kernel_spmd

    if "nc" not in _CACHED:
        _CACHED["nc"] = _build()
    nc = _CACHED["nc"]

    in_maps = _make_in_maps(S, Z, X)
    res = run_bass_kernel_spmd(nc, in_maps, core_ids=list(range(N_CORES)))
    return _finale(res.results)


if __name__ == "__main__":
    import reference
    inputs = reference.setup_inputs()
    inputs = {k: np.asarray(v) for k, v in inputs.items()}
    print("kernel:", kernel(**inputs))


# revision 25
# speedup vs baseline: 1.2394x; 1.0347x over previous
"""Trainium2 8-core kernel for nn_ACCSLP_59485297050024.

The reference is a multiplicative-update NMF-style solver on N=4096 nodes with
rank R=128 and N_ITERS=2, returning a scalar objective O.

Because U, H, W, V are initialized to all-ones (per the problem's input spec),
every multiplicative update keeps each factor CONSTANT along the rank axis, so
the whole computation collapses exactly to rank-1 vector recurrences:

    u1 = (rowsum(S) + b*rowsum(Z)) * 2/(3R)
    h1 = (S + a*X)^T (1/e1) / R,  e1 = u1 + a       v1 = Z^T (1/u1) / R
    w1 = X (1/h1) / R,   u2 = (S + b*Z)(1/d1) / R,  d1 = h1 + b*v1
    h2 = (S + a*X)^T (1/e2) / R,  e2 = u2 + a*w1    v2 = Z^T (1/u2) / R
    w2 = X (1/h2) / R
    O  = R[Su2 Sh2 + a Sw2 Sh2 + b Su2 Sv2]
         - (sum(S) + a sum(X) + b sum(Z)) log R
         - <log u2, rsS + b rsZ> - a <log w2, rsX>
         - <log h2, csS + a csX> - b <log v2, csZ>

(verified exact vs the reference, rel err ~2e-16 in float64).

S only ever appears combined: P = S + a*X (h updates, colsums) and
Q = S + b*Z (u updates), so the device streams FOUR matrices (P, Z row-major;
Q, X col-major), not six.

Device strategy (8 NeuronCores): row-shard the matrices (512 rows/core); keep
row-major shards of P/Z and column-major (transposed) shards of Q/X resident
in SBUF (bf16). Every contraction is a TensorE matmul with a tiny stationary
vector:
  - "B" passes (contract over rows -> h/v/colsums) use row-major tiles;
    partials are AllReduce-summed across cores (2 AllReduces total).
  - "A" passes (contract over cols -> u/w/rowsums) use col-major tiles;
    results stay core-local (each core owns its 512 rows of u/w).
A dummy AllReduce issued first absorbs the per-execution collective warmup and
core start skew under the DMA load phase; chained dummy matmuls keep the PE
clock warm across the real AllReduce waits. The final scalar assembly (logs +
dot products on 4096-vectors) runs on host in float64.
"""

import numpy as np
import ml_dtypes

N = 4096
R = 128
ALPHA = 0.5
BETA = 0.5
N_CORES = 8
RPC = N // N_CORES          # rows per core = 512
RG = RPC // 128             # row groups per core = 4
NC_CH = N // 128            # 128-column chunks = 32
NJ8 = N // 512              # 512-column chunks = 8

_CACHED = {}


def _build():
    import concourse.mybir as mybir
    import concourse.tile as tile
    from concourse import bacc
    from concourse.masks import make_identity

    bf16 = mybir.dt.bfloat16
    f32 = mybir.dt.float32

    nc = bacc.Bacc("TRN2", target_bir_lowering=False, debug=False,
                   num_devices=N_CORES, dynamic_dma_scratch_size=8192)

    # per-core external I/O (host supplies per-partition-contiguous layouts)
    rp_e = nc.declare_dram_parameter("rp", [128, RG, N], bf16, isOutput=False)
    rz_e = nc.declare_dram_parameter("rz", [128, RG, N], bf16, isOutput=False)
    cq_e = nc.declare_dram_parameter("cq", [128, NC_CH, RPC], bf16, isOutput=False)
    cx_e = nc.declare_dram_parameter("cx", [128, NC_CH, RPC], bf16, isOutput=False)
    out_u1 = nc.declare_dram_parameter("u1", [1, RPC], f32, isOutput=True)
    out_u2 = nc.declare_dram_parameter("u2", [1, RPC], f32, isOutput=True)
    out_w2 = nc.declare_dram_parameter("w2", [1, RPC], f32, isOutput=True)
    out_rsx = nc.declare_dram_parameter("rsx", [1, RPC], f32, isOutput=True)
    out_h2 = nc.declare_dram_parameter("h2", [NC_CH, 128], f32, isOutput=True)
    out_v2 = nc.declare_dram_parameter("v2", [NC_CH, 128], f32, isOutput=True)
    out_cssx = nc.declare_dram_parameter("cssx", [NC_CH, 128], f32, isOutput=True)
    out_csz = nc.declare_dram_parameter("csz", [NC_CH, 128], f32, isOutput=True)

    ar1_out = nc.dram_tensor("ar1_out", [2, NC_CH, 128], f32, addr_space="Shared")
    ar2_out = nc.dram_tensor("ar2_out", [2, NC_CH, 128], f32, addr_space="Shared")
    groups = [list(range(N_CORES))]

    with tile.TileContext(nc) as tc:
        with (
            tc.tile_pool(name="big", bufs=1) as big,
            tc.tile_pool(name="small", bufs=1) as small,
            tc.tile_pool(name="stg", bufs=2) as stg,
            tc.tile_pool(name="pp", bufs=1, space="PSUM") as pp,
            tc.tile_pool(name="pstream", bufs=2, space="PSUM") as pstream,
            tc.tile_pool(name="ptrans", bufs=1, space="PSUM") as ptrans,
            tc.tile_pool(name="pwarm", bufs=1, space="PSUM") as pwarm,
            tc.tile_pool(name="dram", bufs=1, space="DRAM") as dram,
        ):
            # ---------- cross-core barrier first ----------
            # dummy AllReduce: absorbs per-execution collective warmup + core
            # start skew while the DMA loads run. Input is never written
            # (garbage) and output never read -- must not wait on anything.
            sync_in = nc.dram_tensor("sync_in", [1, 128], f32)
            sync_out = nc.dram_tensor("sync_out", [1, 128], f32, addr_space="Shared")
            with tc.high_priority():
                nc.gpsimd.collective_compute(
                    "AllReduce", mybir.AluOpType.add, replica_groups=groups,
                    ins=[sync_in[:].opt()], outs=[sync_out[:].opt()])

            # ---------- resident loads (pieces, ordered for pipelining) ----------
            QC = 4   # col-tile pieces per matrix (8 chunks each)
            QR = 4   # row-tile pieces per matrix (1024 cols each)
            W4 = N // QR
            tCq4 = [big.tile([128, 8, RPC], bf16, name=f"tC_q{q}", tag=f"tC_q{q}") for q in range(QC)]
            tCx4 = [big.tile([128, 8, RPC], bf16, name=f"tC_x{q}", tag=f"tC_x{q}") for q in range(QC)]
            tRp4 = [big.tile([128, RG, W4], bf16, name=f"tR_p{q}", tag=f"tR_p{q}") for q in range(QR)]
            tRz4 = [big.tile([128, RG, W4], bf16, name=f"tR_z{q}", tag=f"tR_z{q}") for q in range(QR)]

            def tC(pieces, c):
                return pieces[c // 8][:, c % 8, :]

            def tR(pieces, a, c8):
                w = (c8 % 2) * RPC
                return pieces[c8 // 2][:, a, w:w + RPC]

            # cols of Q first (P0), then rows interleaved (B1), then cols of X
            for q in range(QC):
                nc.sync.dma_start(tCq4[q][:], cq_e[:, q * 8:(q + 1) * 8, :])
            for q in range(QR):
                nc.sync.dma_start(tRp4[q][:], rp_e[:, :, q * W4:(q + 1) * W4])
                nc.sync.dma_start(tRz4[q][:], rz_e[:, :, q * W4:(q + 1) * W4])
            for q in range(QC):
                nc.sync.dma_start(tCx4[q][:], cx_e[:, q * 8:(q + 1) * 8, :])

            ident = small.tile([128, 128], f32, tag="ident")
            make_identity(nc, ident[:])
            onesb = small.tile([128, 2], bf16, tag="onesb")
            nc.gpsimd.memset(onesb[:, 0:1], 1.0)
            nc.gpsimd.memset(onesb[:, 1:2], BETA)

            wsink = small.tile([1, 64], f32, tag="wsink")

            def warm_chain(n):
                """Chained dummy matmuls to keep the PE clock (HAM) warm while
                waiting on a collective. Each link gates the next through a
                ScalarE copy, spreading PE activity over the wait."""
                for _ in range(n):
                    wp = pwarm.tile([1, RPC], f32, tag="warm")
                    nc.tensor.matmul(wp[:], onesb[:, 0:1], tCq4[0][:, 0, :],
                                     start=True, stop=True)
                    nc.scalar.copy(wsink[:], wp[0:1, 0:64])

            # ---------- P0: u1 = rowsum(Q) * 2/(3R) ----------
            ps_p0 = pp.tile([1, RPC], f32, tag="accA")
            for c in range(NC_CH):
                nc.tensor.matmul(ps_p0[:], onesb[:, 0:1], tC(tCq4, c),
                                 start=(c == 0), stop=(c == NC_CH - 1))
            u1sb = small.tile([1, RPC], f32, tag="u1sb")
            nc.vector.tensor_scalar_mul(u1sb[:], ps_p0[:], 2.0 / (3.0 * R))
            nc.sync.dma_start(out_u1[:], u1sb[:])

            # ---------- B1 stationaries: yP1 = 1/(R(u1+a)), yZ1 = 1/(R u1) ----
            y1P = small.tile([1, RPC], f32, tag="yP")
            y1Z = small.tile([1, RPC], f32, tag="yZ")
            t1 = small.tile([1, RPC], f32, tag="t1")
            nc.vector.tensor_scalar(t1[:], u1sb[:], ALPHA, float(R),
                                    mybir.AluOpType.add, mybir.AluOpType.mult)
            nc.vector.reciprocal(y1P[:], t1[:])
            nc.vector.reciprocal(y1Z[:], u1sb[:])   # 1/u1; 1/R folded in copy

            def transpose_rows_to_cols(yvecs):
                """list of [1, 512] f32 -> psum [128, nvec*RG] grouped per vector."""
                ps_t = ptrans.tile([128, 96], f32, tag="pt")
                for v, yv in enumerate(yvecs):
                    for a in range(RG):
                        nc.tensor.transpose(
                            ps_t[:, v * RG + a: v * RG + a + 1],
                            yv[0:1, a * 128:(a + 1) * 128],
                            ident[0:1, 0:1],
                        )
                return ps_t

            ps_t1 = transpose_rows_to_cols([y1P, y1Z])
            statP1 = small.tile([128, 2, RG], bf16, tag="statP1")
            statZ1 = small.tile([128, 2, RG], bf16, tag="statZ1")
            nc.gpsimd.memset(statP1[:, 1, :], 1.0)
            nc.gpsimd.memset(statZ1[:, 1, :], 1.0)
            nc.vector.tensor_copy(statP1[:, 0, :], ps_t1[:, 0:RG])
            nc.vector.tensor_scalar_mul(statZ1[:, 0, :], ps_t1[:, RG:2 * RG], 1.0 / R)

            # ---------- B passes (contract rows) ----------
            # AR1 carries only h1pre,v1pre (critical); colsums ride a separate
            # non-critical AllReduce issued at the end.
            ar1_in = dram.tile([2, NC_CH, 128], f32, tag="ar1_in")
            arcs_in = dram.tile([2, NC_CH, 128], f32, tag="arcs_in")
            arcs_out = nc.dram_tensor("arcs_out", [2, NC_CH, 128], f32, addr_space="Shared")

            def b_pass(statP, statZ, nv, ar_in_t):
                for c8 in range(NJ8):
                    ps_h = pstream.tile([2, RPC], f32, tag="sx")
                    ps_v = pstream.tile([2, RPC], f32, tag="z")
                    for a in range(RG):
                        nc.tensor.matmul(ps_h[0:nv, :],
                                         statP[:, :, a] if nv == 2 else statP[:, a:a + 1],
                                         tR(tRp4, a, c8), start=(a == 0), stop=(a == RG - 1))
                    for a in range(RG):
                        nc.tensor.matmul(ps_v[0:nv, :],
                                         statZ[:, :, a] if nv == 2 else statZ[:, a:a + 1],
                                         tR(tRz4, a, c8), start=(a == 0), stop=(a == RG - 1))
                    st = stg.tile([34, RG, 128], f32, tag="st")
                    nc.scalar.copy(st[0:nv].rearrange("p a b -> p (a b)"), ps_h[0:nv, :])
                    nc.vector.tensor_copy(st[32:32 + nv].rearrange("p a b -> p (a b)"), ps_v[0:nv, :])
                    cs = slice(c8 * RG, (c8 + 1) * RG)
                    nc.gpsimd.dma_start(ar_in_t[0, cs, :], st[0:1])
                    nc.gpsimd.dma_start(ar_in_t[1, cs, :], st[32:33])
                    if nv == 2:
                        nc.gpsimd.dma_start(arcs_in[0, cs, :], st[1:2])
                        nc.gpsimd.dma_start(arcs_in[1, cs, :], st[33:34])

            b_pass(statP1, statZ1, 2, ar1_in)
            nc.gpsimd.collective_compute(
                "AllReduce", mybir.AluOpType.add, replica_groups=groups,
                ins=[ar1_in.opt()], outs=[ar1_out[:].opt()])
            warm_chain(20)

            # ---------- A2 stationaries from AR1 ----------
            # ar1 rows: 0 = h1pre, 1 = v1pre
            h1 = small.tile([NC_CH, 128], f32, tag="h1")
            v1 = small.tile([NC_CH, 128], f32, tag="v1")
            nc.sync.dma_start(h1[:], ar1_out[0])
            nc.sync.dma_start(v1[:], ar1_out[1])
            vstk = small.tile([64, 128], f32, tag="vstk")
            tmp32 = small.tile([NC_CH, 128], f32, tag="tmp32")
            # vstk[0:32] = 1/(h1 + b*v1)  (Q stationary; 1/R folded in copy)
            # vstk[32:64] = 1/h1           (X stationary, with raw ones col)
            nc.vector.scalar_tensor_tensor(tmp32[:], v1[:], BETA, h1[:],
                                           mybir.AluOpType.mult, mybir.AluOpType.add)
            nc.vector.reciprocal(vstk[0:32, :], tmp32[:])
            nc.vector.reciprocal(vstk[32:64, :], h1[:])
            ps_t2 = ptrans.tile([128, 96], f32, tag="pt")
            nc.tensor.transpose(ps_t2[:, 0:64], vstk[:], ident[0:64, 0:64])
            statA_q = small.tile([128, NC_CH], bf16, tag="statA_q")
            statA_x = small.tile([128, 2, NC_CH], bf16, tag="statA_x")
            nc.vector.tensor_scalar_mul(statA_q[:], ps_t2[:, 0:NC_CH], 1.0 / R)
            nc.gpsimd.memset(statA_x[:, 1, :], 1.0)
            nc.vector.tensor_scalar_mul(statA_x[:, 0, :], ps_t2[:, NC_CH:2 * NC_CH], 1.0 / R)

            # ---------- A2 (contract cols): u2 | w1, rsX ----------
            ps_a2u = pp.tile([1, RPC], f32, tag="accA")
            ps_a2wx = pp.tile([2, RPC], f32, tag="accB")
            for c in range(NC_CH):
                nc.tensor.matmul(ps_a2u[:], statA_q[:, c:c + 1], tC(tCq4, c),
                                 start=(c == 0), stop=(c == NC_CH - 1))
            for c in range(NC_CH):
                nc.tensor.matmul(ps_a2wx[:], statA_x[:, :, c], tC(tCx4, c),
                                 start=(c == 0), stop=(c == NC_CH - 1))
            a2u = small.tile([1, RPC], f32, tag="a2u")
            a2wx = small.tile([2, RPC], f32, tag="a2wx")
            nc.scalar.copy(a2u[:], ps_a2u[:])
            nc.scalar.copy(a2wx[:], ps_a2wx[:])
            nc.sync.dma_start(out_u2[:], a2u[:])
            nc.sync.dma_start(out_rsx[:], a2wx[1:2, :])

            # ---------- B2 stationaries: yP2 = 1/(R(u2+a w1)), yZ2 = 1/(R u2)
            y2P = small.tile([1, RPC], f32, tag="yP")
            y2Z = small.tile([1, RPC], f32, tag="yZ")
            t2 = small.tile([1, RPC], f32, tag="t1")
            nc.vector.scalar_tensor_tensor(t2[:], a2wx[0:1, :], ALPHA, a2u[:],
                                           mybir.AluOpType.mult, mybir.AluOpType.add)
            nc.vector.reciprocal(y2P[:], t2[:])
            nc.vector.reciprocal(y2Z[:], a2u[:])
            ps_t3 = transpose_rows_to_cols([y2P, y2Z])
            statP2 = small.tile([128, RG], bf16, tag="statP2")
            statZ2 = small.tile([128, RG], bf16, tag="statZ2")
            nc.vector.tensor_scalar_mul(statP2[:], ps_t3[:, 0:RG], 1.0 / R)
            nc.vector.tensor_scalar_mul(statZ2[:], ps_t3[:, RG:2 * RG], 1.0 / R)

            # ---------- B2 -> AR2 (rows: h2pre, v2pre) ----------
            ar2_in = dram.tile([2, NC_CH, 128], f32, tag="ar2_in")
            b_pass(statP2, statZ2, 1, ar2_in)
            nc.gpsimd.collective_compute(
                "AllReduce", mybir.AluOpType.add, replica_groups=groups,
                ins=[ar2_in.opt()], outs=[ar2_out[:].opt()])
            nc.gpsimd.collective_compute(
                "AllReduce", mybir.AluOpType.add, replica_groups=groups,
                ins=[arcs_in.opt()], outs=[arcs_out[:].opt()])

            # ---------- A3: w2 = X (1/h2) / R ----------
            h2 = small.tile([NC_CH, 128], f32, tag="h1")
            nc.sync.dma_start(h2[:], ar2_out[0])
            nc.vector.reciprocal(vstk[0:32, :], h2[:])
            ps_t4 = ptrans.tile([128, 96], f32, tag="pt")
            nc.tensor.transpose(ps_t4[:, 0:32], vstk[0:32, :], ident[0:32, 0:32])
            statA3 = small.tile([128, NC_CH], bf16, tag="statA3")
            nc.vector.tensor_scalar_mul(statA3[:], ps_t4[:, 0:32], 1.0 / R)
            ps_a3 = pp.tile([2, RPC], f32, tag="accB")
            for c in range(NC_CH):
                nc.tensor.matmul(ps_a3[0:1, :], statA3[:, c:c + 1], tC(tCx4, c),
                                 start=(c == 0), stop=(c == NC_CH - 1))
            w2sb = small.tile([1, RPC], f32, tag="u1sb")
            nc.scalar.copy(w2sb[:], ps_a3[0:1, :])
            nc.sync.dma_start(out_w2[:], w2sb[:])

            # ---------- replicated outputs (DRAM->DRAM) ----------
            nc.sync.dma_start(out_h2[:], ar2_out[0])
            nc.sync.dma_start(out_v2[:], ar2_out[1])
            nc.sync.dma_start(out_cssx[:], arcs_out[0])
            nc.sync.dma_start(out_csz[:], arcs_out[1])

    nc.compile()
    return nc


def _finale(res):
    """Assemble the scalar objective from per-core device outputs (float64)."""
    u1 = np.concatenate([np.asarray(res[i]["u1"], np.float64).ravel() for i in range(N_CORES)])
    u2 = np.concatenate([np.asarray(res[i]["u2"], np.float64).ravel() for i in range(N_CORES)])
    w2 = np.concatenate([np.asarray(res[i]["w2"], np.float64).ravel() for i in range(N_CORES)])
    rsx = np.concatenate([np.asarray(res[i]["rsx"], np.float64).ravel() for i in range(N_CORES)])
    h2 = np.asarray(res[0]["h2"], np.float64).ravel()
    v2 = np.asarray(res[0]["v2"], np.float64).ravel()
    cssx = np.asarray(res[0]["cssx"], np.float64).ravel()
    csz = np.asarray(res[0]["csz"], np.float64).ravel()

    rs_sz = u1 * (1.5 * R)          # = rsS + b*rsZ  (u1 = rs_sz*2/(3R))
    lR = np.log(R)
    term1 = R * (u2.sum() * h2.sum() + ALPHA * w2.sum() * h2.sum()
                 + BETA * u2.sum() * v2.sum())
    # sum(S) + a*sum(X) + b*sum(Z) = sum(cssx) + b*sum(csz)
    O = (term1
         - (cssx.sum() + BETA * csz.sum()) * lR
         - (np.log(u2) * rs_sz).sum()
         - ALPHA * (np.log(w2) * rsx).sum()
         - (np.log(h2) * cssx).sum()
         - BETA * (np.log(v2) * csz).sum())
    return np.float32(O)


def _numpy_fallback(S, Z, X, U, H, W, V):
    """Faithful CPU implementation (only used if factors are not all-ones)."""
    S, Z, X, U, H, W, V = [np.asarray(a, np.float32) for a in (S, Z, X, U, H, W, V)]

    def obj(Sp, Xp, Zp):
        return ((Sp - S * np.log(Sp)).sum()
                + ALPHA * (Xp - X * np.log(Xp)).sum()
                + BETA * (Zp - Z * np.log(Zp)).sum())

    Sp = U @ H; Xp = W @ H; Zp = U @ V
    Sd = S / Sp; Xd = X / Xp; Zd = Z / Zp
    O = obj(Sp, Xp, Zp)
    for _ in range(2):
        dHV = H + BETA * V
        U = U * (Sd @ (H / dHV).T + Zd @ ((BETA * V) / dHV).T)
        Sp = U @ H; Zp = U @ V; Sd = S / Sp; Zd = Z / Zp
        dUW = U + ALPHA * W
        H = H * ((U / dUW).T @ Sd + ((ALPHA * W) / dUW).T @ Xd)
        Sp = U @ H; Xp = W @ H; Sd = S / Sp; Xd = X / Xp
        W = W * Xd.sum(axis=1, keepdims=True)
        Xp = W @ H; Xd = X / Xp
        V = V * Zd.sum(axis=0, keepdims=True)
        Zp = U @ V; Zd = Z / Zp
        O = obj(Sp, Xp, Zp)
    return np.float32(O)


def _make_in_maps(S, Z, X):
    S = np.asarray(S, np.float32)
    Z = np.asarray(Z, np.float32)
    X = np.asarray(X, np.float32)
    P = (S + ALPHA * X).astype(ml_dtypes.bfloat16)   # h-side combined matrix
    Q = (S + BETA * Z).astype(ml_dtypes.bfloat16)    # u-side combined matrix
    Xb = X.astype(ml_dtypes.bfloat16)
    Zb = Z.astype(ml_dtypes.bfloat16)

    def row_layout(shard):
        # [512, 4096] -> [128(p), RG(a), 4096(j)], per-partition contiguous
        return np.ascontiguousarray(shard.reshape(RG, 128, N).transpose(1, 0, 2))

    def col_layout(shard):
        # [512, 4096] -> [128(p), NC_CH(c), 512(l)] where (c,p) indexes column j
        return np.ascontiguousarray(
            shard.T.reshape(NC_CH, 128, RPC).transpose(1, 0, 2))

    in_maps = []
    for c in range(N_CORES):
        rows = slice(c * RPC, (c + 1) * RPC)
        in_maps.append({
            "rp": row_layout(P[rows]), "rz": row_layout(Zb[rows]),
            "cq": col_layout(Q[rows]), "cx": col_layout(Xb[rows]),
        })
    return in_maps


def kernel(S, Z, X, U, H, W, V):
    if not (np.all(np.asarray(U) == 1) and np.all(np.asarray(H) == 1)
            and np.all(np.asarray(W) == 1) and np.all(np.asarray(V) == 1)):
        return _numpy_fallback(S, Z, X, U, H, W, V)

    from concourse.bass_utils import run_bass_kernel_spmd

    if "nc" not in _CACHED:
        _CACHED["nc"] = _build()
    nc = _CACHED["nc"]

    in_maps = _make_in_maps(S, Z, X)
    res = run_bass_kernel_spmd(nc, in_maps, core_ids=list(range(N_CORES)))
    return _finale(res.results)


if __name__ == "__main__":
    import reference
    inputs = reference.setup_inputs()
    inputs = {k: np.asarray(v) for k, v in inputs.items()}
    print("kernel:", kernel(**inputs))
